# revision 1
# baseline (speedup 1.0000x reference)
"""ContrastiveLoss kernel for 8 Trainium2 NeuronCores (Bass/Tile).

Computes  mean over (label==dep_idx, label==sui_idx) pairs of
          relu(MARGIN - ||(e_i + eps) - e_j||)
for embeddings [4096, 768] f32, labels [4096] int.

Strategy (data-parallel 4x2 grid over the 4096x4096 pair matrix):
  - core c handles d-rows R*1024:(R+1)*1024 (R=c//2) and s-cols
    C*2048:(C+1)*2048 (C=c%2).
  - dist2 = ||d_i||^2 + ||s_j||^2 - 2 d_i.s_j is computed as ONE bf16
    GEMM with the contraction dim augmented by 2 rows:
        u_i = (d_i, a_i, 1),  v_j = (-2 s_j, 1, b_j)
        a_i = ||d_i||^2 + BIG*(1 - dep_i),  b_j = ||s_j||^2 + BIG*(1 - sui_j)
    so u.v = dist2 + BIG*(masked-out terms): masked-out pairs get dist
    pushed far above MARGIN, making relu(MARGIN - dist) exactly 0 there.
    => no per-element masking needed after the GEMM.
  - epilogue: ACT sqrt (PSUM -> bf16), DVE min(dist-1, 0) + row-sum.
  - per-core outputs [sum(min(dist-1,0)), n_dep_local, n_sui_local];
    host combines: total = -sum(p0), count = sum(p1*p2), loss = total/count.

Host does ONLY fixed-range slicing and int64->int32 label staging; all
math (masks, norms, casts, transposes, GEMM, hinge, reductions) is on
device.
"""

import os
import sys

import numpy as np

# B x D embeddings, 4x2 core grid.
B, D = 4096, 768
GR, GC = 4, 2
M_LOC, N_LOC = B // GR, B // GC  # 1024 d-rows, 2048 s-cols per core
P = 128
KT = D // P  # 6 K-tiles of 128
MT = M_LOC // P  # 8 M-tiles
NT = N_LOC // P  # 16 N-subtiles
NCH = 4  # N chunks per core (PSUM tiles of [128, 512])
CH = N_LOC // NCH  # 512
MARGIN = 1.0
EPS = 1e-6
BIG = 100.0  # pushes masked-out pairs' dist2 >= ~BIG >> MARGIN^2

_REPO = "/opt/trn_rl_repo"

_cache: dict = {}


def _ensure_import():
    try:
        import concourse.bass  # noqa: F401
    except ModuleNotFoundError:
        sys.path.insert(0, _REPO)


def _emit(tc, nc, dep_idx: float, sui_idx: float, reps: int = 1):
    import concourse.bass as bass
    import concourse.mybir as mybir

    f32 = mybir.dt.float32
    i32 = mybir.dt.int32

    lhs_d = nc.dram_tensor("lhs", [M_LOC, D], f32, kind="ExternalInput")
    rhs_d = nc.dram_tensor("rhs", [N_LOC, D], f32, kind="ExternalInput")
    labm_d = nc.dram_tensor("labm", [M_LOC], i32, kind="ExternalInput")
    labn_d = nc.dram_tensor("labn", [N_LOC], i32, kind="ExternalInput")
    out_d = nc.dram_tensor("partials", [3], f32, kind="ExternalOutput")
    for _ in range(reps):
        _emit_body(tc, nc, dep_idx, sui_idx, lhs_d, rhs_d, labm_d, labn_d, out_d)


_POOL_SEQ = [0]


def _emit_body(tc, nc, dep_idx, sui_idx, lhs_d, rhs_d, labm_d, labn_d, out_d):
    import concourse.bass as bass
    import concourse.mybir as mybir
    from concourse.masks import make_identity

    f32 = mybir.dt.float32
    bf16 = mybir.dt.bfloat16
    i32 = mybir.dt.int32
    AF = mybir.ActivationFunctionType
    ALU = mybir.AluOpType
    PSUM = bass.MemorySpace.PSUM
    _POOL_SEQ[0] += 1
    u = f"_{_POOL_SEQ[0]}"

    with (
        tc.tile_pool(name="const" + u, bufs=1) as constp,
        tc.tile_pool(name="stage_f" + u, bufs=4) as stagef,
        tc.tile_pool(name="stage_b" + u, bufs=6) as stageb,
        tc.tile_pool(name="sq" + u, bufs=2) as sqp,
        tc.tile_pool(name="kmaj" + u, bufs=1) as kmaj,
        tc.tile_pool(name="dist" + u, bufs=2) as distp,
        tc.tile_pool(name="small" + u, bufs=1) as small,
        tc.tile_pool(name="ptp" + u, bufs=2, space=PSUM) as ptp,
        tc.tile_pool(name="pmm" + u, bufs=6, space=PSUM) as pmm,
    ):
        # ---- constants & labels -------------------------------------
        ident = constp.tile([P, P], bf16)
        make_identity(nc, ident[:])
        ones_f = constp.tile([P, 1], f32)
        nc.vector.memset(ones_f[:], 1.0)
        eps_bias = constp.tile([P, 1], f32)
        nc.vector.memset(eps_bias[:], EPS)
        zero_bias = constp.tile([P, 1], f32)
        nc.vector.memset(zero_bias[:], 0.0)

        labm_sb = small.tile([P, MT], i32)
        nc.sync.dma_start(
            out=labm_sb[:], in_=labm_d[:].rearrange("(t p) -> p t", p=P)
        )
        labn_sb = small.tile([P, NT], i32)
        nc.sync.dma_start(
            out=labn_sb[:], in_=labn_d[:].rearrange("(t p) -> p t", p=P)
        )
        labm_f = small.tile([P, MT], f32)
        nc.vector.tensor_copy(labm_f[:], labm_sb[:])
        labn_f = small.tile([P, NT], f32)
        nc.vector.tensor_copy(labn_f[:], labn_sb[:])
        dep = small.tile([P, MT], f32)
        nc.vector.tensor_scalar(
            out=dep[:], in0=labm_f[:], scalar1=float(dep_idx), scalar2=None,
            op0=ALU.is_equal,
        )
        sui = small.tile([P, NT], f32)
        nc.vector.tensor_scalar(
            out=sui[:], in0=labn_f[:], scalar1=float(sui_idx), scalar2=None,
            op0=ALU.is_equal,
        )
        ndep = small.tile([P, 1], f32)
        nc.vector.tensor_reduce(
            out=ndep[:], in_=dep[:], axis=mybir.AxisListType.X, op=ALU.add
        )
        nsui = small.tile([P, 1], f32)
        nc.vector.tensor_reduce(
            out=nsui[:], in_=sui[:], axis=mybir.AxisListType.X, op=ALU.add
        )

        # per-partition stats
        d2 = small.tile([P, MT], f32)   # ||d_i||^2 per lhs tile column
        s2 = small.tile([P, NT], f32)   # ||s_j||^2 per rhs tile column
        hsum = small.tile([P, NCH * MT], f32)  # hinge row-sums per psum tile

        # K-major (transposed) bf16 operands. kb axis: 0..5 main, 6 aug.
        lhsT = [kmaj.tile([P, KT + 1, P], bf16, tag=f"lhsT{m}", name=f"lhsT{m}") for m in range(MT)]
        rhsT = [
            kmaj.tile([P, KT + 1, CH], bf16, tag=f"rhsT{c}", name=f"rhsT{c}")
            for c in range(NCH)
        ]

        def prep_tile(t, is_lhs):
            """DMA one [128, 768] f32 row-tile, cast to bf16 (+aug cols),
            PE-transpose into the K-major operand tiles."""
            src = lhs_d if is_lhs else rhs_d
            stg = stagef.tile([P, D], f32, tag="stg_f")
            nc.sync.dma_start(out=stg[:], in_=src[t * P : (t + 1) * P, :])

            stgb = stageb.tile([P, D + 2], bf16, tag="stg_b")
            if is_lhs:
                # d = emb + EPS; s-side scaled by -2 so u.v = -2 d.s + a + b
                nc.vector.tensor_scalar(
                    out=stgb[:, 0:D], in0=stg[:], scalar1=EPS, scalar2=None,
                    op0=ALU.add,
                )
            else:
                nc.vector.tensor_scalar(
                    out=stgb[:, 0:D], in0=stg[:], scalar1=-2.0, scalar2=None,
                    op0=ALU.mult,
                )

            # per-row squared norm via ACT Square + free-axis accumulate
            sqt = sqp.tile([P, D], bf16, tag="sqt")
            acc = (d2 if is_lhs else s2)[:, t : t + 1]
            nc.scalar.activation(
                out=sqt[:], in_=stg[:], func=AF.Square,
                bias=(eps_bias[:] if is_lhs else zero_bias[:]), scale=1.0,
                accum_out=acc,
            )

            # aug value: a = d2 + BIG*(1-dep)  (lhs) / b = s2 + BIG*(1-sui)
            mask_col = (dep if is_lhs else sui)[:, t : t + 1]
            avar = stagef.tile([P, 1], f32, tag="avar")
            nc.vector.tensor_scalar(
                out=avar[:], in0=mask_col, scalar1=-BIG, scalar2=BIG,
                op0=ALU.mult, op1=ALU.add,
            )
            nc.vector.tensor_tensor(
                out=avar[:], in0=avar[:], in1=acc, op=ALU.add
            )
            # aug columns: lhs -> (a, 1);  rhs -> (1, b)
            if is_lhs:
                nc.vector.tensor_copy(stgb[:, D : D + 1], avar[:])
                nc.vector.memset(stgb[:, D + 1 : D + 2], 1.0)
            else:
                nc.vector.memset(stgb[:, D : D + 1], 1.0)
                nc.vector.tensor_copy(stgb[:, D + 1 : D + 2], avar[:])

            # transpose 6x [128,128] + 1x [128,2] into K-major tiles
            if is_lhs:
                dst, doff = lhsT[t], 0
            else:
                dst, doff = rhsT[t // 4], (t % 4) * P

            pa = ptp.tile([P, 4 * P], bf16, tag="tp")
            for kb in range(4):
                nc.tensor.transpose(
                    pa[:, kb * P : (kb + 1) * P],
                    stgb[:, kb * P : (kb + 1) * P],
                    ident[:],
                )
            nc.vector.tensor_copy(
                dst[:, 0:4, doff : doff + P],
                pa[:].rearrange("p (k x) -> p k x", k=4),
            )
            pb = ptp.tile([P, 3 * P], bf16, tag="tp")
            for kb in (4, 5):
                nc.tensor.transpose(
                    pb[:, (kb - 4) * P : (kb - 3) * P],
                    stgb[:, kb * P : (kb + 1) * P],
                    ident[:],
                )
            nc.tensor.transpose(
                pb[0:2, 2 * P : 3 * P], stgb[:, D : D + 2], ident[:]
            )
            nc.vector.tensor_copy(
                dst[:, 4:6, doff : doff + P],
                pb[:, 0 : 2 * P].rearrange("p (k x) -> p k x", k=2),
            )
            nc.vector.tensor_copy(
                dst[0:2, 6, doff : doff + P], pb[0:2, 2 * P : 3 * P]
            )

        dists = {}

        def main_chunk(ch):
            for m in range(MT):
                ps = pmm.tile([P, CH], f32, tag="mm")
                for kb in range(KT):
                    nc.tensor.matmul(
                        ps[:],
                        lhsT[m][:, kb, :],
                        rhsT[ch][:, kb, :],
                        start=(kb == 0),
                        stop=False,
                    )
                nc.tensor.matmul(
                    ps[:],
                    lhsT[m][0:2, 6, :],
                    rhsT[ch][0:2, 6, :],
                    start=False,
                    stop=True,
                )
                dist = distp.tile(
                    [P, CH], bf16, tag=f"dist{ch}_{m}", name=f"dist{ch}_{m}"
                )
                nc.scalar.activation(
                    out=dist[:], in_=ps[:], func=AF.Sqrt, bias=zero_bias[:]
                )
                dists[(ch, m)] = dist

        def hinge_pass():
            # batched after all sqrts: one Relu table load, accumulate
            # sum(relu(MARGIN - dist)) per psum tile on ACT
            for ch in range(NCH):
                for m in range(MT):
                    trash = distp.tile([P, CH], bf16, tag="trash")
                    nc.scalar.activation(
                        out=trash[:], in_=dists[(ch, m)][:], func=AF.Relu,
                        bias=ones_f[:], scale=-1.0,
                        accum_out=hsum[:, ch * MT + m : ch * MT + m + 1],
                    )

        # phase 1: all preps (ACT=Square only); phase 2: dense matmul+sqrt
        # block (PE warm, ACT=Sqrt only); phase 3: batched Relu+accum.
        for m in range(MT):
            prep_tile(m, True)
        for t in range(NT):
            prep_tile(t, False)
        for ch in range(NCH):
            main_chunk(ch)
        hinge_pass()

        # ---- final: pack [hinge_sum, ndep, nsui] and partition-sum ----
        hrow = small.tile([P, 1], f32)
        nc.vector.tensor_reduce(
            out=hrow[:], in_=hsum[:], axis=mybir.AxisListType.X, op=ALU.add
        )
        nc.vector.tensor_scalar_mul(hrow[:], hrow[:], -1.0)
        pack = small.tile([P, 3], f32)
        nc.vector.tensor_copy(pack[:, 0:1], hrow[:])
        nc.vector.tensor_copy(pack[:, 1:2], ndep[:])
        nc.vector.tensor_copy(pack[:, 2:3], nsui[:])
        stats_ps = ptp.tile([3, 1], f32, tag="tp")
        nc.tensor.matmul(stats_ps[:], pack[:], ones_f[:], start=True, stop=True)
        out_sb = small.tile([3, 1], f32)
        nc.scalar.copy(out_sb[:], stats_ps[:])
        nc.sync.dma_start(out=out_d[:], in_=out_sb[:, 0])


CAPM = 256  # compact dep capacity per core (actual <= 177 per quarter)
CAPN = 384  # compact sui capacity per core (actual <= 343 per half)
GM = CAPM // P
GN = CAPN // P
BIGP = 1000.0  # pad-slot dist^2 boost; kills pad contributions exactly


def _emit_body_gather(tc, nc, dep_idx, sui_idx, lhs_d, rhs_d, labm_d, labn_d,
                      out_d):
    """Compact variant via SWDGE dma_gather.

    On-device: label masks -> per-slot ranks (scan) -> compact row indices
    (one-hot f32 matmuls, one [1,cap] psum row per side) -> int16 idx tile
    (both sides packed, 16-wrapped, replicated across the 8 GPSIMD windows
    by 4 doubling DMAs) -> dma_gather of ONLY dep/sui rows -> bf16 cast +
    PE transpose to K-major -> 6-chunk bf16 GEMM; ||d||^2 and ||s||^2 are
    added into PSUM by two K=1 matmuls (outer-product adds), with +BIGP on
    pad slots -> ACT sqrt -> ACT relu(1-d) row sums. Pad slots gather row 0
    and are killed by BIGP.
    """
    import concourse.bass as bass
    import concourse.mybir as mybir
    from concourse.masks import make_identity

    f32 = mybir.dt.float32
    bf16 = mybir.dt.bfloat16
    i32 = mybir.dt.int32
    i16 = mybir.dt.int16
    AF = mybir.ActivationFunctionType
    ALU = mybir.AluOpType
    PSUM = bass.MemorySpace.PSUM
    X = mybir.AxisListType.X
    _POOL_SEQ[0] += 1
    u = f"_{_POOL_SEQ[0]}"

    CAPX = max(CAPM, CAPN)
    CAPA = CAPM + CAPN
    iota_np = np.tile(np.arange(CAPX, dtype=np.float32), (P, 1))
    iota_d = nc.inline_tensor(iota_np, name="iotag" + u)
    # rid lhsT per row-tile t: col0 = p, col1 = t (both bf16-exact)
    NTX = max(MT, NT)
    rid_np = np.zeros((P, NTX, 2), dtype=np.float32)
    rid_np[:, :, 0] = np.arange(P, dtype=np.float32)[:, None]
    rid_np[:, :, 1] = np.arange(NTX, dtype=np.float32)[None, :]
    rid_d = nc.inline_tensor(rid_np, name="ridg" + u)
    wvec_d = nc.inline_tensor(np.array([[1.0], [128.0]], dtype=np.float32),
                              name="wvecg" + u)
    sltri_d = nc.inline_tensor(
        np.triu(np.ones((P, P), dtype=np.float32), k=1), name="sltrig" + u)

    with (
        tc.tile_pool(name="const" + u, bufs=1) as constp,
        tc.tile_pool(name="oh" + u, bufs=4) as ohp,
        tc.tile_pool(name="ga" + u, bufs=1) as gap,
        tc.tile_pool(name="stage" + u, bufs=3) as stagep,
        tc.tile_pool(name="kmaj" + u, bufs=1) as kmaj,
        tc.tile_pool(name="sqk" + u, bufs=2) as sqkp,
        tc.tile_pool(name="dist" + u, bufs=2) as distp,
        tc.tile_pool(name="small" + u, bufs=1) as small,
        tc.tile_pool(name="ptp" + u, bufs=2, space=PSUM) as ptp,
        tc.tile_pool(name="pix" + u, bufs=1, space=PSUM) as pix,
        tc.tile_pool(name="pnr" + u, bufs=1, space=PSUM) as pnr,
        tc.tile_pool(name="pmm" + u, bufs=2, space=PSUM) as pmm,
    ):
        # ---- constants ----------------------------------------------
        ident = constp.tile([P, P], bf16)
        make_identity(nc, ident[:])
        ident_f = constp.tile([P, P], f32)
        make_identity(nc, ident_f[:])
        ones_f = constp.tile([P, 1], f32)
        nc.vector.memset(ones_f[:], 1.0)
        ones_colb = constp.tile([P, 1], bf16)
        nc.vector.memset(ones_colb[:], 1.0)
        ones_rowb = constp.tile([1, CAPX], bf16)
        nc.vector.memset(ones_rowb[:], 1.0)
        zero_bias = constp.tile([P, 1], f32)
        nc.vector.memset(zero_bias[:], 0.0)
        zeros_nt = constp.tile([P, NTX], f32)
        nc.vector.memset(zeros_nt[:], 0.0)
        zrow = constp.tile([1, P], f32)
        nc.vector.memset(zrow[:], 0.0)
        iota_sb = constp.tile([P, CAPX], f32)
        nc.sync.dma_start(out=iota_sb[:], in_=iota_d[:])
        rid_f = constp.tile([P, NTX, 2], f32)
        nc.sync.dma_start(out=rid_f[:], in_=rid_d[:])
        rid_b = constp.tile([P, NTX, 2], bf16)
        nc.vector.tensor_copy(rid_b[:], rid_f[:])
        wvec = constp.tile([2, 1], f32)
        nc.sync.dma_start(out=wvec[:], in_=wvec_d[:])
        sltri = constp.tile([P, P], f32)
        nc.sync.dma_start(out=sltri[:], in_=sltri_d[:])

        # ---- per-side ranks (masks + scans) -------------------------
        def ranks(lab_d, nt, idx_val, cap, name):
            lab_sb = small.tile([P, nt], i32, name=f"lab_{name}")
            nc.sync.dma_start(
                out=lab_sb[:], in_=lab_d[:].rearrange("(t p) -> p t", p=P)
            )
            labf = small.tile([P, nt], f32, name=f"labf_{name}")
            nc.vector.tensor_copy(labf[:], lab_sb[:])
            mask = small.tile([P, nt], f32, name=f"mask_{name}")
            nc.vector.tensor_scalar(
                out=mask[:], in0=labf[:], scalar1=float(idx_val), scalar2=None,
                op0=ALU.is_equal,
            )
            cnt = small.tile([P, 1], f32, name=f"cnt_{name}")
            nc.vector.tensor_reduce(out=cnt[:], in_=mask[:], axis=X, op=ALU.add)
            # offs[p] = sum_{p'<p} cnt[p'] and total via K=128 matmuls
            # (strict-lower-tri inline const; avoids K=1 matmuls + scans)
            ops_ = ptp.tile([P, 1], f32, tag="tp", name=f"ops_{name}")
            nc.tensor.matmul(ops_[:], sltri[:], cnt[:], start=True, stop=True)
            offs = small.tile([P, 1], f32, name=f"offs_{name}")
            nc.vector.tensor_copy(offs[:], ops_[:])
            tot_ps = ptp.tile([1, 1], f32, tag="tp", name=f"totp_{name}")
            nc.tensor.matmul(tot_ps[:], cnt[:], ones_f[:, 0:1],
                             start=True, stop=True)
            total = small.tile([1, 1], f32, name=f"tot_{name}")
            nc.vector.tensor_copy(total[:], tot_ps[:])
            incl = small.tile([P, nt], f32, name=f"incl_{name}")
            nc.vector.tensor_tensor_scan(
                out=incl[:], data0=mask[:], data1=zeros_nt[:, 0:nt],
                initial=0.0, op0=ALU.add, op1=ALU.add,
            )
            rank = small.tile([P, nt], f32, name=f"rank_{name}")
            nc.vector.tensor_sub(rank[:], incl[:], mask[:])
            nc.vector.tensor_scalar(
                out=rank[:], in0=rank[:], scalar1=offs[:, 0:1], scalar2=None,
                op0=ALU.add,
            )
            # unmasked rows -> sentinel rank = cap (outside iota range)
            pen = small.tile([P, nt], f32, name=f"pen_{name}")
            nc.vector.tensor_scalar(
                out=pen[:], in0=mask[:], scalar1=-float(cap),
                scalar2=float(cap), op0=ALU.mult, op1=ALU.add,
            )
            nc.vector.tensor_mul(rank[:], rank[:], mask[:])
            nc.vector.tensor_add(rank[:], rank[:], pen[:])
            return rank, cnt, total

        rank_n, cnt_n, tot_n = ranks(labn_d, NT, sui_idx, CAPN, "n")
        rank_m, cnt_m, tot_m = ranks(labm_d, MT, dep_idx, CAPM, "m")

        # ---- compact idx rows via interleaved one-hot matmuls -------
        ix_n = pix.tile([2, CAPN], f32, tag="ixn", name="ixn")
        ix_m = pix.tile([2, CAPM], f32, tag="ixm", name="ixm")

        def oh_mm(rank, ix, cap, t, nt, name):
            oh = ohp.tile([P, cap], bf16, tag=f"oh_{name}")
            nc.vector.tensor_scalar(
                out=oh[:], in0=iota_sb[:, 0:cap],
                scalar1=rank[:, t : t + 1], scalar2=None, op0=ALU.is_equal,
            )
            nc.tensor.matmul(
                ix[:], rid_b[:, t, :], oh[:],
                start=(t == 0), stop=(t == nt - 1),
            )

        for t in range(NT):
            oh_mm(rank_n, ix_n, CAPN, t, NT, "n")
            if t < MT:
                oh_mm(rank_m, ix_m, CAPM, t, MT, "m")

        # combine idx = row0 + 128*row1 via one K=2 matmul per side
        ixs_n = small.tile([2, CAPN], f32, name="ixs_n")
        nc.vector.tensor_copy(ixs_n[:], ix_n[:])
        ixs_m = small.tile([2, CAPM], f32, name="ixs_m")
        nc.vector.tensor_copy(ixs_m[:], ix_m[:])
        ixr_n = pix.tile([1, CAPN], f32, tag="ixn", name="cb_n")
        nc.tensor.matmul(ixr_n[:], wvec[:], ixs_n[:], start=True, stop=True)
        ixr_m = pix.tile([1, CAPM], f32, tag="ixm", name="cb_m")
        nc.tensor.matmul(ixr_m[:], wvec[:], ixs_m[:], start=True, stop=True)

        # ---- pack idx, wrap to 16, replicate x8 (per side: the Q7
        # gather ucode assumes the idx tile row pitch == num_idxs/16) ----
        idx16_m = small.tile([1, CAPM], i16, name="idx16m")
        nc.vector.tensor_copy(idx16_m[:], ixr_m[:])
        idx16_n = small.tile([1, CAPN], i16, name="idx16n")
        nc.vector.tensor_copy(idx16_n[:], ixr_n[:])

        def wrap_rep(sl, cap, name):
            w = small.tile([P, cap // 16], i16, name=f"idxw_{name}")
            nc.sync.dma_start(
                out=w[0:16, :], in_=sl.rearrange("a (c q) -> (a q) c", q=16)
            )
            nc.sync.dma_start(out=w[16:32, :], in_=w[0:16, :])
            nc.sync.dma_start(out=w[32:64, :], in_=w[0:32, :])
            nc.sync.dma_start(out=w[64:128, :], in_=w[0:64, :])
            return w

        idxw_n = wrap_rep(idx16_n[:], CAPN, "n")
        idxw_m = wrap_rep(idx16_m[:], CAPM, "m")

        # ---- gathers ------------------------------------------------
        ga_n = gap.tile([P, GN, D], f32, name="ga_n")
        nc.gpsimd.dma_gather(ga_n[:], rhs_d[:], idxw_n[:], CAPN, CAPN, D)
        ga_m = gap.tile([P, GM, D], f32, name="ga_m")
        nc.gpsimd.dma_gather(ga_m[:], lhs_d[:], idxw_m[:], CAPM, CAPM, D)

        # ---- cast + K-major transpose -------------------------------
        lhsT = [
            kmaj.tile([P, KT, P], bf16, tag=f"glT{g}", name=f"glT{g}")
            for g in range(GM)
        ]
        rhsT = kmaj.tile([P, KT, CAPN], bf16, name="grT")

        def prep(j, is_lhs):
            stg = (ga_m if is_lhs else ga_n)[:, j, :]
            stgb = stagep.tile([P, D], bf16, tag="stgb")
            if is_lhs:
                # u = -2 (d + eps) so u.v = -2 d.s; norms rescaled by 1/4
                nc.vector.tensor_scalar(
                    out=stgb[:], in0=stg, scalar1=EPS, scalar2=-2.0,
                    op0=ALU.add, op1=ALU.mult,
                )
            else:
                nc.vector.tensor_copy(stgb[:], stg)
            dst = lhsT[j] if is_lhs else rhsT
            doff = 0 if is_lhs else j * P
            pa = ptp.tile([P, 4 * P], bf16, tag="tp")
            for kb in range(4):
                nc.tensor.transpose(
                    pa[:, kb * P : (kb + 1) * P],
                    stgb[:, kb * P : (kb + 1) * P],
                    ident[:],
                )
            nc.vector.tensor_copy(
                dst[:, 0:4, doff : doff + P],
                pa[:].rearrange("p (k x) -> p k x", k=4),
            )
            pb = ptp.tile([P, 2 * P], bf16, tag="tp")
            for kb in (4, 5):
                nc.tensor.transpose(
                    pb[:, (kb - 4) * P : (kb - 3) * P],
                    stgb[:, kb * P : (kb + 1) * P],
                    ident[:],
                )
            nc.vector.tensor_copy(
                dst[:, 4:6, doff : doff + P],
                pb[:].rearrange("p (k x) -> p k x", k=2),
            )

        for j in range(GN):
            prep(j, False)
        for j in range(GM):
            prep(j, True)

        # ---- norms as [1, cap] rows: DVE square + ones matmuls ------
        def norm_row(kt_tiles, cap, total, scale, name):
            nps = pnr.tile([1, cap], f32, tag=f"nr_{name}", name=f"nr_{name}")
            n_mm = len(kt_tiles) * KT
            i = 0
            for tile_ in kt_tiles:
                w = tile_.shape[2]
                sq = sqkp.tile([P, KT, w], bf16, tag=f"sq_{name}")
                nc.vector.tensor_mul(sq[:], tile_[:], tile_[:])
                for kc in range(KT):
                    off = 0 if len(kt_tiles) == 1 else kt_tiles.index(tile_) * w
                    nc.tensor.matmul(
                        nps[0:1, off : off + w], ones_colb[:], sq[:, kc, :],
                        start=(kc == 0), stop=(kc == KT - 1),
                    )
                    i += 1
            # aug row: scale*norms + BIGP on pad slots (slot >= total)
            padr = small.tile([1, cap], f32, name=f"padr_{name}")
            nc.vector.tensor_scalar(
                out=padr[:], in0=iota_sb[0:1, 0:cap],
                scalar1=total[0:1, 0:1], scalar2=float(BIGP),
                op0=ALU.is_ge, op1=ALU.mult,
            )
            arow = small.tile([1, cap], bf16, name=f"arow_{name}")
            nc.vector.tensor_scalar(
                out=arow[:], in0=nps[:], scalar1=scale, scalar2=None,
                op0=ALU.mult,
            )
            nc.vector.tensor_add(arow[:], arow[:], padr[:])
            return arow

        brow = norm_row([rhsT], CAPN, tot_n, 1.0, "n")
        arow = norm_row(lhsT, CAPM, tot_m, 0.25, "m")

        # ---- pair GEMM (+ K=2 zero-padded outer-product adds) -------
        aadd = small.tile([2, CAPM], bf16, name="aadd")
        nc.vector.memset(aadd[:], 0.0)
        nc.vector.tensor_copy(aadd[0:1, :], arow[:])
        badd = small.tile([2, CAPN], bf16, name="badd")
        nc.vector.memset(badd[:], 0.0)
        nc.vector.tensor_copy(badd[0:1, :], brow[:])
        ones2 = small.tile([2, CAPX], bf16, name="ones2")
        nc.vector.memset(ones2[:], 1.0)
        onesel = small.tile([2, P], bf16, name="onesel")
        nc.vector.memset(onesel[:], 0.0)
        nc.vector.memset(onesel[0:1, :], 1.0)
        hsum = small.tile([P, GM], f32)
        for g in range(GM):
            ps = pmm.tile([P, CAPN], f32, tag="mm")
            for kb in range(KT):
                nc.tensor.matmul(
                    ps[:], lhsT[g][:, kb, :], rhsT[:, kb, :],
                    start=(kb == 0), stop=False,
                )
            # += a_i (dep norms, varies along partitions)
            nc.tensor.matmul(
                ps[:], aadd[:, g * P : (g + 1) * P],
                ones2[:, 0:CAPN], start=False, stop=False,
            )
            # += b_j (sui norms, varies along free axis)
            nc.tensor.matmul(
                ps[:], onesel[:, 0:P], badd[:, 0:CAPN],
                start=False, stop=True,
            )
            dist = distp.tile([P, CAPN], bf16, tag="dist")
            nc.scalar.activation(
                out=dist[:], in_=ps[:], func=AF.Sqrt, bias=zero_bias[:]
            )
            trash = distp.tile([P, CAPN], bf16, tag="trash")
            nc.scalar.activation(
                out=trash[:], in_=dist[:], func=AF.Relu,
                bias=ones_f[:], scale=-1.0,
                accum_out=hsum[:, g : g + 1],
            )

        # ---- pack [hinge_sum, ndep, nsui], partition-sum, store -----
        hrow = small.tile([P, 1], f32)
        nc.vector.tensor_reduce(out=hrow[:], in_=hsum[:], axis=X, op=ALU.add)
        pack = small.tile([P, 3], f32)
        nc.vector.tensor_copy(pack[:, 0:1], hrow[:])
        nc.vector.tensor_copy(pack[:, 1:2], cnt_m[:])
        nc.vector.tensor_copy(pack[:, 2:3], cnt_n[:])
        stats_ps = ptp.tile([3, 1], f32, tag="tp")
        nc.tensor.matmul(stats_ps[:], pack[:], ones_f[:], start=True, stop=True)
        out_sb = small.tile([3, 1], f32)
        nc.scalar.copy(out_sb[:], stats_ps[:])
        nc.sync.dma_start(out=out_d[:], in_=out_sb[:, 0])


def _build_gather(dep_idx, sui_idx):
    key = ("gather", float(dep_idx), float(sui_idx))
    if key in _cache:
        return _cache[key]
    _ensure_import()
    import concourse.mybir as mybir
    import concourse.tile as tile
    from concourse import bacc

    nc = bacc.Bacc("TRN2", target_bir_lowering=False, debug=False)
    with tile.TileContext(nc) as tc:
        f32 = mybir.dt.float32
        i32 = mybir.dt.int32
        lhs_d = nc.dram_tensor("lhs", [M_LOC, D], f32, kind="ExternalInput")
        rhs_d = nc.dram_tensor("rhs", [N_LOC, D], f32, kind="ExternalInput")
        labm_d = nc.dram_tensor("labm", [M_LOC], i32, kind="ExternalInput")
        labn_d = nc.dram_tensor("labn", [N_LOC], i32, kind="ExternalInput")
        out_d = nc.dram_tensor("partials", [3], f32, kind="ExternalOutput")
        _emit_body_gather(
            tc, nc, float(dep_idx), float(sui_idx),
            lhs_d, rhs_d, labm_d, labn_d, out_d,
        )
    nc.compile()
    _cache[key] = nc
    return nc


def _build(dep_idx, sui_idx, reps: int = 1):
    key = (float(dep_idx), float(sui_idx), reps)
    if key in _cache:
        return _cache[key]
    _ensure_import()
    import concourse.tile as tile
    from concourse import bacc

    nc = bacc.Bacc("TRN2", target_bir_lowering=False, debug=False)
    with tile.TileContext(nc) as tc:
        _emit(tc, nc, key[0], key[1], reps=reps)
    nc.compile()
    _cache[key] = nc
    return nc


NPM = 384  # padded compact dep-row capacity per core (expect ~171)
NPN = 512  # padded compact sui-row capacity per core (expect ~341)
HUGE = 1_000_000


def _emit_body_compact(tc, nc, dep_idx, sui_idx, lhs_d, rhs_d, labm_d, labn_d,
                       out_d):
    """Compacted variant: gather only label==dep rows (lhs) and label==sui
    rows (rhs) via on-device rank/scatter/gather, then run the small GEMM
    on padded [NPM] x [NPN] compact sets. Pad slots stay all-zero and get
    +BIG in the augmented column => contribute exactly 0."""
    import concourse.bass as bass
    import concourse.mybir as mybir
    from concourse.masks import make_identity

    f32 = mybir.dt.float32
    bf16 = mybir.dt.bfloat16
    i32 = mybir.dt.int32
    AF = mybir.ActivationFunctionType
    ALU = mybir.AluOpType
    PSUM = bass.MemorySpace.PSUM
    X = mybir.AxisListType.X
    _POOL_SEQ[0] += 1
    u = f"_{_POOL_SEQ[0]}"

    RPM = NPM // P  # compact lhs rows per partition (3)
    RPN = NPN // P  # compact rhs rows per partition (4)

    rid_m_np = np.arange(M_LOC, dtype=np.int32).reshape(MT, P).T.copy()
    rid_n_np = np.arange(N_LOC, dtype=np.int32).reshape(NT, P).T.copy()
    rid_m_d = nc.inline_tensor(rid_m_np, name="rid_m" + u)
    rid_n_d = nc.inline_tensor(rid_n_np, name="rid_n" + u)

    with (
        tc.tile_pool(name="const" + u, bufs=1) as constp,
        tc.tile_pool(name="cstage" + u, bufs=1) as cstage,
        tc.tile_pool(name="stage_b" + u, bufs=4) as stageb,
        tc.tile_pool(name="sq" + u, bufs=2) as sqp,
        tc.tile_pool(name="kmaj" + u, bufs=1) as kmaj,
        tc.tile_pool(name="dist" + u, bufs=2) as distp,
        tc.tile_pool(name="small" + u, bufs=1) as small,
        tc.tile_pool(name="dram" + u, bufs=1, space="DRAM") as dramp,
        tc.tile_pool(name="ptp" + u, bufs=2, space=PSUM) as ptp,
        tc.tile_pool(name="pmm" + u, bufs=3, space=PSUM) as pmm,
    ):
        # ---- constants -----------------------------------------------
        ident = constp.tile([P, P], bf16)
        make_identity(nc, ident[:])
        ident_f = constp.tile([P, P], f32)
        make_identity(nc, ident_f[:])
        ones_f = constp.tile([P, 1], f32)
        nc.vector.memset(ones_f[:], 1.0)
        eps_bias = constp.tile([P, 1], f32)
        nc.vector.memset(eps_bias[:], EPS)
        zero_bias = constp.tile([P, 1], f32)
        nc.vector.memset(zero_bias[:], 0.0)
        zeros_nt = constp.tile([P, NT], f32)
        nc.vector.memset(zeros_nt[:], 0.0)
        zrow = constp.tile([1, P], f32)
        nc.vector.memset(zrow[:], 0.0)

        labm_sb = small.tile([P, MT], i32)
        nc.sync.dma_start(
            out=labm_sb[:], in_=labm_d[:].rearrange("(t p) -> p t", p=P)
        )
        labn_sb = small.tile([P, NT], i32)
        nc.sync.dma_start(
            out=labn_sb[:], in_=labn_d[:].rearrange("(t p) -> p t", p=P)
        )
        rid_m = small.tile([P, MT], i32)
        nc.sync.dma_start(out=rid_m[:], in_=rid_m_d[:])
        rid_n = small.tile([P, NT], i32)
        nc.sync.dma_start(out=rid_n[:], in_=rid_n_d[:])

        # ---- masks + per-partition counts ----------------------------
        def build_mask(lab_sb, nt, idx_val, name):
            labf = small.tile([P, nt], f32, name=f"labf_{name}")
            nc.vector.tensor_copy(labf[:], lab_sb[:])
            mask = small.tile([P, nt], f32, name=f"mask_{name}")
            nc.vector.tensor_scalar(
                out=mask[:], in0=labf[:], scalar1=float(idx_val), scalar2=None,
                op0=ALU.is_equal,
            )
            cnt = small.tile([P, 1], f32, name=f"cnt_{name}")
            nc.vector.tensor_reduce(out=cnt[:], in_=mask[:], axis=X, op=ALU.add)
            return mask, cnt

        dep, ndep = build_mask(labm_sb, MT, dep_idx, "m")
        sui, nsui = build_mask(labn_sb, NT, sui_idx, "n")

        # ---- ranks: offs[p] + exclusive scan along free --------------
        def build_scatter_idx(mask, cnt, nt, rid, cap, name):
            # partition-axis exclusive offsets via transpose+scan+transpose
            cps = ptp.tile([1, P], f32, tag="tp", name=f"cps_{name}")
            nc.tensor.matmul(cps[:], cnt[:], ident_f[:], start=True, stop=True)
            crow = small.tile([1, P], f32, name=f"crow_{name}")
            nc.vector.tensor_copy(crow[:], cps[:])
            srow = small.tile([1, P], f32, name=f"srow_{name}")
            nc.vector.tensor_tensor_scan(
                out=srow[:], data0=crow[:], data1=zrow[:], initial=0.0,
                op0=ALU.add, op1=ALU.add,
            )
            orow = small.tile([1, P], f32, name=f"orow_{name}")
            nc.vector.tensor_sub(orow[:], srow[:], crow[:])
            ops_ = ptp.tile([P, 1], f32, tag="tp", name=f"ops_{name}")
            nc.tensor.matmul(
                ops_[:], orow[0:1, :], ones_f[0:1, 0:1], start=True, stop=True
            )
            offs = small.tile([P, 1], f32, name=f"offs_{name}")
            nc.vector.tensor_copy(offs[:], ops_[:])

            incl = small.tile([P, nt], f32, name=f"incl_{name}")
            nc.vector.tensor_tensor_scan(
                out=incl[:], data0=mask[:], data1=zeros_nt[:, 0:nt],
                initial=0.0, op0=ALU.add, op1=ALU.add,
            )
            rank = small.tile([P, nt], f32, name=f"rank_{name}")
            nc.vector.tensor_sub(rank[:], incl[:], mask[:])
            nc.vector.tensor_scalar(
                out=rank[:], in0=rank[:], scalar1=offs[:, 0:1], scalar2=None,
                op0=ALU.add,
            )
            # masked-out rows -> HUGE (dropped by scatter bounds check)
            pen = small.tile([P, nt], f32, name=f"pen_{name}")
            nc.vector.tensor_scalar(
                out=pen[:], in0=mask[:], scalar1=-float(HUGE),
                scalar2=float(HUGE), op0=ALU.mult, op1=ALU.add,
            )
            nc.vector.tensor_mul(rank[:], rank[:], mask[:])
            nc.vector.tensor_add(rank[:], rank[:], pen[:])
            scat = small.tile([P, nt], i32, name=f"scat_{name}")
            nc.vector.tensor_copy(scat[:], rank[:])

            # scatter row-ids into DRAM index array (prefilled with HUGE)
            idx_dram = dramp.tile([cap], i32, name=f"idxd_{name}")
            sent = small.tile([P, cap // P], i32, name=f"sent_{name}")
            nc.vector.memset(sent[:], HUGE)
            nc.sync.dma_start(
                out=idx_dram[:].rearrange("(p r) -> p r", p=P), in_=sent[:]
            )
            nc.gpsimd.indirect_dma_start(
                out=idx_dram[:].rearrange("(a b) -> a b", b=1),
                out_offset=bass.IndirectOffsetOnAxis(ap=scat[:], axis=0),
                in_=rid[:],
                in_offset=None,
                bounds_check=cap - 1,
                oob_is_err=False,
            )
            idx_sb = small.tile([P, cap // P], i32, name=f"idxs_{name}")
            nc.sync.dma_start(
                out=idx_sb[:], in_=idx_dram[:].rearrange("(p r) -> p r", p=P)
            )
            return idx_sb

        scat_m = build_scatter_idx(dep, ndep, MT, rid_m, NPM, "m")
        scat_n = build_scatter_idx(sui, nsui, NT, rid_n, NPN, "n")

        # ---- gather compact rows ------------------------------------
        def gather_rows(idx_sb, src_d, rpp, nrows, name):
            comp = cstage.tile([P, rpp, D], f32, name=f"comp_{name}")
            nc.vector.memset(comp[:], 0.0)
            nc.gpsimd.indirect_dma_start(
                out=comp[:],
                out_offset=None,
                in_=src_d[:],
                in_offset=bass.IndirectOffsetOnAxis(ap=idx_sb[:], axis=0),
                bounds_check=nrows - 1,
                oob_is_err=False,
            )
            padf = small.tile([P, rpp], f32, name=f"padf_{name}")
            nc.vector.tensor_copy(padf[:], idx_sb[:])
            nc.vector.tensor_scalar(
                out=padf[:], in0=padf[:], scalar1=float(HUGE) / 2.0,
                scalar2=None, op0=ALU.is_ge,
            )
            return comp, padf

        comp_m, pad_m = gather_rows(scat_m, lhs_d, RPM, M_LOC, "m")
        comp_n, pad_n = gather_rows(scat_n, rhs_d, RPN, N_LOC, "n")

        # ---- compact prep: cast/aug/transpose -----------------------
        lhsTc = [
            kmaj.tile([P, KT + 1, P], bf16, tag=f"lhsTc{m}", name=f"lhsTc{m}")
            for m in range(RPM)
        ]
        rhsTc = kmaj.tile([P, KT + 1, NPN], bf16, name="rhsTc")
        d2c = small.tile([P, RPM], f32)
        s2c = small.tile([P, RPN], f32)

        def prep_compact(r, is_lhs):
            stg = (comp_m if is_lhs else comp_n)[:, r, :]
            stgb = stageb.tile([P, D + 2], bf16, tag="stg_b")
            if is_lhs:
                nc.vector.tensor_scalar(
                    out=stgb[:, 0:D], in0=stg, scalar1=EPS, scalar2=None,
                    op0=ALU.add,
                )
            else:
                nc.vector.tensor_scalar(
                    out=stgb[:, 0:D], in0=stg, scalar1=-2.0, scalar2=None,
                    op0=ALU.mult,
                )
            sqt = sqp.tile([P, D], bf16, tag="sqt")
            acc = (d2c if is_lhs else s2c)[:, r : r + 1]
            nc.scalar.activation(
                out=sqt[:], in_=stg, func=AF.Square,
                bias=(eps_bias[:] if is_lhs else zero_bias[:]), scale=1.0,
                accum_out=acc,
            )
            # aug = ||row||^2 + BIG * is_pad
            pcol = (pad_m if is_lhs else pad_n)[:, r : r + 1]
            avar = small.tile([P, 1], f32, name=f"avc_{is_lhs}_{r}")
            nc.vector.tensor_scalar(
                out=avar[:], in0=pcol, scalar1=BIG, scalar2=None, op0=ALU.mult
            )
            nc.vector.tensor_add(avar[:], avar[:], acc)
            if is_lhs:
                nc.vector.tensor_copy(stgb[:, D : D + 1], avar[:])
                nc.vector.memset(stgb[:, D + 1 : D + 2], 1.0)
            else:
                nc.vector.memset(stgb[:, D : D + 1], 1.0)
                nc.vector.tensor_copy(stgb[:, D + 1 : D + 2], avar[:])

            dst = lhsTc[r] if is_lhs else rhsTc
            doff = 0 if is_lhs else r * P
            pa = ptp.tile([P, 4 * P], bf16, tag="tp")
            for kb in range(4):
                nc.tensor.transpose(
                    pa[:, kb * P : (kb + 1) * P],
                    stgb[:, kb * P : (kb + 1) * P],
                    ident[:],
                )
            nc.vector.tensor_copy(
                dst[:, 0:4, doff : doff + P],
                pa[:].rearrange("p (k x) -> p k x", k=4),
            )
            pb = ptp.tile([P, 3 * P], bf16, tag="tp")
            for kb in (4, 5):
                nc.tensor.transpose(
                    pb[:, (kb - 4) * P : (kb - 3) * P],
                    stgb[:, kb * P : (kb + 1) * P],
                    ident[:],
                )
            nc.tensor.transpose(
                pb[0:2, 2 * P : 3 * P], stgb[:, D : D + 2], ident[:]
            )
            nc.vector.tensor_copy(
                dst[:, 4:6, doff : doff + P],
                pb[:, 0 : 2 * P].rearrange("p (k x) -> p k x", k=2),
            )
            nc.vector.tensor_copy(
                dst[0:2, 6, doff : doff + P], pb[0:2, 2 * P : 3 * P]
            )

        for r in range(RPN):
            prep_compact(r, False)
        for r in range(RPM):
            prep_compact(r, True)

        # ---- compact GEMM + epilogue --------------------------------
        hsum = small.tile([P, RPM], f32)
        for m in range(RPM):
            ps = pmm.tile([P, NPN], f32, tag="mm")
            for kb in range(KT):
                nc.tensor.matmul(
                    ps[:], lhsTc[m][:, kb, :], rhsTc[:, kb, :],
                    start=(kb == 0), stop=False,
                )
            nc.tensor.matmul(
                ps[:], lhsTc[m][0:2, 6, :], rhsTc[0:2, 6, :],
                start=False, stop=True,
            )
            dist = distp.tile([P, NPN], bf16, tag="dist")
            nc.scalar.activation(
                out=dist[:], in_=ps[:], func=AF.Sqrt, bias=zero_bias[:]
            )
            hng = distp.tile([P, NPN], bf16, tag="hng")
            nc.vector.tensor_scalar(
                out=hng[:], in0=dist[:], scalar1=MARGIN, scalar2=0.0,
                op0=ALU.subtract, op1=ALU.min,
            )
            nc.vector.tensor_reduce(
                out=hsum[:, m : m + 1], in_=hng[:], axis=X, op=ALU.add
            )

        hrow = small.tile([P, 1], f32)
        nc.vector.tensor_reduce(out=hrow[:], in_=hsum[:], axis=X, op=ALU.add)
        pack = small.tile([P, 3], f32)
        nc.vector.tensor_copy(pack[:, 0:1], hrow[:])
        nc.vector.tensor_copy(pack[:, 1:2], ndep[:])
        nc.vector.tensor_copy(pack[:, 2:3], nsui[:])
        stats_ps = ptp.tile([3, 1], f32, tag="tp")
        nc.tensor.matmul(stats_ps[:], pack[:], ones_f[:], start=True, stop=True)
        out_sb = small.tile([3, 1], f32)
        nc.scalar.copy(out_sb[:], stats_ps[:])
        nc.sync.dma_start(out=out_d[:], in_=out_sb[:, 0])


def _build_compact(dep_idx, sui_idx):
    key = ("compact", float(dep_idx), float(sui_idx))
    if key in _cache:
        return _cache[key]
    _ensure_import()
    import concourse.mybir as mybir
    import concourse.tile as tile
    from concourse import bacc

    nc = bacc.Bacc("TRN2", target_bir_lowering=False, debug=False)
    with tile.TileContext(nc) as tc:
        f32 = mybir.dt.float32
        i32 = mybir.dt.int32
        lhs_d = nc.dram_tensor("lhs", [M_LOC, D], f32, kind="ExternalInput")
        rhs_d = nc.dram_tensor("rhs", [N_LOC, D], f32, kind="ExternalInput")
        labm_d = nc.dram_tensor("labm", [M_LOC], i32, kind="ExternalInput")
        labn_d = nc.dram_tensor("labn", [N_LOC], i32, kind="ExternalInput")
        out_d = nc.dram_tensor("partials", [3], f32, kind="ExternalOutput")
        _emit_body_compact(
            tc, nc, float(dep_idx), float(sui_idx),
            lhs_d, rhs_d, labm_d, labn_d, out_d,
        )
    nc.compile()
    _cache[key] = nc
    return nc


def _build_loop(dep_idx, sui_idx, n_iters: int, body: str = "full"):
    """Bench-only: body wrapped in a HW For_i loop (n_iters iterations)."""
    key = ("loop", body, float(dep_idx), float(sui_idx), n_iters)
    if key in _cache:
        return _cache[key]
    _ensure_import()
    import concourse.mybir as mybir
    import concourse.tile as tile
    from concourse import bacc

    emit = {"full": _emit_body, "compact": _emit_body_compact,
            "gather": _emit_body_gather}[body]
    nc = bacc.Bacc("TRN2", target_bir_lowering=False, debug=False)
    with tile.TileContext(nc) as tc:
        f32 = mybir.dt.float32
        i32 = mybir.dt.int32
        lhs_d = nc.dram_tensor("lhs", [M_LOC, D], f32, kind="ExternalInput")
        rhs_d = nc.dram_tensor("rhs", [N_LOC, D], f32, kind="ExternalInput")
        labm_d = nc.dram_tensor("labm", [M_LOC], i32, kind="ExternalInput")
        labn_d = nc.dram_tensor("labn", [N_LOC], i32, kind="ExternalInput")
        out_d = nc.dram_tensor("partials", [3], f32, kind="ExternalOutput")
        with tc.For_i(
            0, n_iters, 1,
            hint_engines=(mybir.EngineType.PE, mybir.EngineType.DVE),
        ):
            emit(
                tc, nc, float(dep_idx), float(sui_idx),
                lhs_d, rhs_d, labm_d, labn_d, out_d,
            )
    nc.compile()
    _cache[key] = nc
    return nc


LAST_RESULTS = None


def kernel(embeddings, labels, depression_idx, suicidal_idx):
    global LAST_RESULTS
    emb = np.ascontiguousarray(np.asarray(embeddings, dtype=np.float32))
    lab = np.asarray(labels).astype(np.int32)
    assert emb.shape == (B, D), emb.shape

    _ensure_import()
    from concourse.bass_utils import run_bass_kernel_spmd

    in_maps = []
    for c in range(8):
        R, C = divmod(c, GC)
        in_maps.append(
            {
                "lhs": np.ascontiguousarray(emb[R * M_LOC : (R + 1) * M_LOC]),
                "rhs": np.ascontiguousarray(emb[C * N_LOC : (C + 1) * N_LOC]),
                "labm": np.ascontiguousarray(lab[R * M_LOC : (R + 1) * M_LOC]),
                "labn": np.ascontiguousarray(lab[C * N_LOC : (C + 1) * N_LOC]),
            }
        )

    # Production path: the full bf16-GEMM kernel (known-good on HW).
    # The compact SWDGE-gather variant (_build_gather) is CoreSim-exact and
    # ~1.8x faster in TimelineSim but NaNs on HW. Eliminated: fp16 onehot
    # compare, K=1/K=2 outer-product adds, idx tile pitch, K=1 offsets
    # matmul (now tri-matmul). Remaining: the partition-expanding SBUF wrap
    # DMA, PSUM f32->i16 copy, free-axis tensor_tensor_scan. Bisect with
    # ExternalOutput dumps of rank/idx16/idxw/ga vs CoreSim next session.
    nc = _build(depression_idx, suicidal_idx)
    res = run_bass_kernel_spmd(nc, in_maps, list(range(8)), trace=False)
    LAST_RESULTS = res
    parts = [np.asarray(r["partials"], dtype=np.float32) for r in res.results]

    total = np.float32(0.0)
    count = np.float32(0.0)
    for p in parts:
        total = np.float32(total + np.float32(-p[0]))
        count = np.float32(count + np.float32(p[1] * p[2]))
    if count > 0:
        loss = np.float32(total / max(count, np.float32(1.0)))
    else:
        loss = np.float32(0.0)
    return np.asarray(loss, dtype=np.float32)

